# revision 1
# baseline (speedup 1.0000x reference)
"""v25: v24 + av/av/den/den emission order.

Was "v24: v18 + bf16 output stores, half-group O phase.

Was "v18: v10 with LAG=4 (extra exp->AV slack).

Was "v10: v2 + partition-major leading DMA layouts for a ~7us kernel start.

Multi-head attention (B=2,S=2048,E=1024,H=16,D=64) on 8 Trainium2 NeuronCores.

Sharding: token-parallel, zero collectives. Core c owns output tokens
[c*512, (c+1)*512) of the flattened (b, s) stream (cores 0-3 = batch 0,
4-7 = batch 1). Each core computes the full K/V projections for its batch
(replicated across the 4 cores of that batch), Q for its own 512 tokens,
attention over all 2048 keys for all 16 heads, and the output projection —
entirely locally. Host concatenates the 8 token shards. No cross-core
dependency exists, so core exec time is immune to launch skew and no
bootstrap collective barrier is emitted into the NEFF.

All matmul operands are bf16 (full PE rate, FWL weight loads); PSUM
accumulation is fp32. Softmax skips max-subtraction (|scores*scale| < ~4)
and exp runs fp32 on ACT. The softmax denominator is computed by a
ones-stationary matmul that broadcasts sum_k(prob) across 64 partitions,
so no cross-partition broadcast is needed for normalization.

Head-pair structure: heads are processed as 8 pairs (pair hp = heads
2hp, 2hp+1 = feature columns 128hp..128hp+127). Score matmuls for the two
heads of a pair run concurrently via PE row tiling (K=64 each); AV and
denominator matmuls run concurrently via PE col tiling (M=64 each). The
K-projection of pair hp+1 is interleaved into pair hp's attention so the
PE stream stays dense (HAM stays at K=8/8) while ACT drains the exps.
"""

import sys

if "/opt/trn_rl_repo" not in sys.path:
    sys.path.insert(0, "/opt/trn_rl_repo")

import numpy as np

B, S, E, H, D = 2, 2048, 1024, 16, 64
N_CORES = 8
T = B * S                  # 4096 tokens total
TB = S                     # 2048 tokens per batch
TSH = T // N_CORES         # 512 tokens owned per core
NP = H // 2                # 8 head pairs
EC = E // 128              # 8 contraction chunks
NKT = TB // 128            # 16 key tiles per batch
SCALE = float(D) ** -0.5

_NC_CACHE = {}


def _emit_body(nc, tc, d, pools):
    import concourse.mybir as mybir

    f32 = mybir.dt.float32
    bf16 = mybir.dt.bfloat16
    Exp = mybir.ActivationFunctionType.Exp

    wpool, big, kpool, scratch = pools["w"], pools["big"], pools["k"], pools["s"]

    # --- resident inputs ---------------------------------------------------
    x_s = big.tile([128, EC, TB], bf16, tag="x")       # x^T for this batch
    xq_s = big.tile([128, EC, TSH], bf16, tag="xq")    # x^T for own tokens
    wq_s = wpool.tile([128, NP, EC, 128], bf16, tag="wq")    # Wq^T  [i, o]
    wk_s = wpool.tile([128, EC, E], bf16, tag="wk")
    wv_s = wpool.tile([128, EC, E], bf16, tag="wv")
    wo_s = wpool.tile([128, EC, E], bf16, tag="wo")
    bq_s = wpool.tile([128, NP, 1], f32, tag="bq")     # [o-in-tile, o-tile]
    bk_s = wpool.tile([128, NP, 1], f32, tag="bk")
    bvB = wpool.tile([128, E], f32, tag="bvB")         # partition-broadcast
    boB = wpool.tile([128, E], f32, tag="boB")
    ones = wpool.tile([128, D], bf16, tag="ones")

    # DMA order = first-needed first. xqT/wqT arrive partition-major from the
    # host so per-chunk transfers keep >=1KB contiguous runs per partition;
    # the Q projection's first matmul can start ~7us into the kernel instead
    # of waiting ~19us for whole-tensor transfers.
    x_ap = d["xT"].ap().rearrange("(c p) t -> p c t", p=128)
    for ic in range(EC):
        nc.sync.dma_start(out=xq_s[:, ic, :], in_=d["xqT"].ap()[ic])
    for ot in range(NP):
        nc.sync.dma_start(out=wq_s[:, ot], in_=d["wqT"].ap()[ot])
    nc.sync.dma_start(out=bq_s[:].rearrange("p n o -> p (n o)"), in_=d["bq_t"].ap())
    nc.sync.dma_start(out=x_s[:, :, 0:512], in_=x_ap[:, :, 0:512])
    nc.sync.dma_start(out=wv_s[:], in_=d["wvT"].ap().rearrange("(c p) o -> p c o", p=128))
    nc.sync.dma_start(out=bvB[:], in_=d["bvB"].ap())
    for ts_ in range(1, 4):
        nc.sync.dma_start(out=x_s[:, :, ts_ * 512:(ts_ + 1) * 512],
                          in_=x_ap[:, :, ts_ * 512:(ts_ + 1) * 512])
    nc.sync.dma_start(out=wk_s[:], in_=d["wkT"].ap().rearrange("(c p) o -> p c o", p=128))
    nc.sync.dma_start(out=bk_s[:].rearrange("p n o -> p (n o)"), in_=d["bk_t"].ap())
    nc.sync.dma_start(out=ones[:], in_=d["ones"].ap())
    nc.sync.dma_start(out=wo_s[:], in_=d["woT"].ap().rearrange("(c p) o -> p c o", p=128))
    nc.sync.dma_start(out=boB[:], in_=d["boB"].ap())

    # --- persistent activations -------------------------------------------
    qT = big.tile([128, NP, TSH], bf16, tag="qT")      # [d-in-pair, pair, tok]
    vn = big.tile([128, NKT, E], bf16, tag="vn")       # [tok-in-tile, tile, feat]
    attnT = big.tile([128, NP, TSH], bf16, tag="attnT")

    # --- phase Q+V: Q projection (own tokens), V projection (all tokens) ---
    with tc.tile_pool(name="ppA", bufs=4, space="PSUM") as ppA:
        for ot in range(NP):                  # Q: out feature tiles = pairs
            ps = ppA.tile([128, TSH], f32, tag="pA")
            for ic in range(EC):
                nc.tensor.matmul(ps[:], wq_s[:, ot, ic, :],
                                 xq_s[:, ic, :], start=(ic == 0), stop=(ic == EC - 1))
            nc.vector.tensor_add(qT[:, ot, :], ps[:],
                                 bq_s[:, ot, :].broadcast_to((128, TSH)))
        for tcn in range(NKT):                # V: token tiles, x stationary
            pv = {oh: ppA.tile([128, 512], f32, tag="pA", name=f"pv{oh}_{tcn}")
                  for oh in (0, 1)}
            for ic in range(EC):
                for oh in (0, 1):
                    nc.tensor.matmul(pv[oh][:], x_s[:, ic, tcn * 128:(tcn + 1) * 128],
                                     wv_s[:, ic, oh * 512:(oh + 1) * 512],
                                     start=(ic == 0), stop=(ic == EC - 1))
            for oh in (0, 1):
                nc.vector.tensor_add(vn[:, tcn, oh * 512:(oh + 1) * 512], pv[oh][:],
                                     bvB[:, oh * 512:(oh + 1) * 512])

    # --- K projection emitter (pair hp), split into 32 single matmuls ------
    kt_slots = {}

    def k_mm(hp, m, ppK):
        """Emit the m-th (0..31) K-proj matmul for pair hp; DVE drain at strip end."""
        ts, ic = divmod(m, EC)
        tsl = slice(ts * 512, (ts + 1) * 512)
        if ic == 0:
            if hp not in kt_slots:
                kt_slots[hp] = kpool.tile([128, TB], bf16, tag="kT", name=f"kT{hp}")
            kt_slots[(hp, ts)] = ppK.tile([128, 512], f32, tag="pK", name=f"pK{hp}_{ts}")
        ps = kt_slots[(hp, ts)]
        nc.tensor.matmul(ps[:], wk_s[:, ic, hp * 128:(hp + 1) * 128],
                         x_s[:, ic, tsl], start=(ic == 0), stop=(ic == EC - 1))
        if ic == EC - 1:
            nc.vector.tensor_add(kt_slots[hp][:, tsl], ps[:],
                                 bk_s[:, hp, :].broadcast_to((128, 512)))

    # --- pair loop: scores + exp + AV/denominator, K(hp+1) interleaved -----
    LAG = 4
    with tc.tile_pool(name="ppK", bufs=2, space="PSUM") as ppK, \
         tc.tile_pool(name="psc", bufs=2, space="PSUM") as psc, \
         tc.tile_pool(name="pav", bufs=1, space="PSUM") as pav, \
         tc.tile_pool(name="probs", bufs=LAG + 1) as prpool:

        for m in range(4 * EC):               # prologue: all of K(0)
            k_mm(0, m, ppK)

        for hp in range(NP):
            kt = kt_slots[hp]
            probs = {}
            avden = pav.tile([128, 2, 512], f32, tag="avden", name=f"avden{hp}")
            for g in range(NKT + LAG):
                if g < NKT:
                    # scores for both heads of the pair (row-tiled, concurrent)
                    scps = psc.tile([128, 2, 512], f32, tag="sc", name=f"sc{hp}_{g}")
                    for h in (0, 1):
                        nc.tensor.matmul(
                            scps[:, h, :],
                            kt[64 * h:64 * h + 64, g * 128:(g + 1) * 128],
                            qT[64 * h:64 * h + 64, hp, :],
                            start=True, stop=True, tile_position=(64 * h, 0))
                    pr = prpool.tile([128, 2, 512], bf16, tag="pr", name=f"pr{hp}_{g}")
                    nc.scalar.activation(pr[:], scps[:], Exp, scale=SCALE)
                    probs[g] = pr
                    # two K-proj matmuls for the next pair (keeps PE dense)
                    if hp + 1 < NP:
                        k_mm(hp + 1, 2 * g, ppK)
                        k_mm(hp + 1, 2 * g + 1, ppK)
                if g >= LAG:
                    gg = g - LAG
                    pr = probs.pop(gg)
                    for h in (0, 1):      # both AVs adjacent: col groups disjoint
                        nc.tensor.matmul(
                            avden[64 * h:64 * h + 64, 0, :],
                            vn[:, gg, hp * 128 + 64 * h: hp * 128 + 64 * h + 64],
                            pr[:, h, :], start=(gg == 0), stop=(gg == NKT - 1))
                    for h in (0, 1):      # then both DENs
                        nc.tensor.matmul(
                            avden[64 * h:64 * h + 64, 1, :],
                            ones[:], pr[:, h, :],
                            start=(gg == 0), stop=(gg == NKT - 1))
            # normalize: attnT[:, hp, :] = av / denom
            rc = scratch.tile([128, 512], f32, tag="rc", name=f"rc{hp}")
            nc.vector.reciprocal(rc[:], avden[:, 1, :])
            nc.vector.tensor_mul(attnT[:, hp, :], avden[:, 0, :], rc[:])

    # --- output projection: half-groups, bf16 stores (halved tail DMA) -----
    with tc.tile_pool(name="ppO", bufs=4, space="PSUM") as ppO, \
         tc.tile_pool(name="outp", bufs=4) as outpool:
        for tt in range(TSH // 128):
            ops = {oh: ppO.tile([128, 512], f32, tag="pO", name=f"ops{tt}_{oh}")
                   for oh in (0, 1)}
            for ic in range(EC):
                for oh in (0, 1):
                    nc.tensor.matmul(
                        ops[oh][:], attnT[:, ic, tt * 128:(tt + 1) * 128],
                        wo_s[:, ic, oh * 512:(oh + 1) * 512],
                        start=(ic == 0), stop=(ic == EC - 1))
            for oh in (0, 1):
                osl = slice(oh * 512, (oh + 1) * 512)
                ot = outpool.tile([128, 512], bf16, tag="ot", name=f"ot{tt}_{oh}")
                nc.vector.tensor_add(ot[:], ops[oh][:], boB[:, osl])
                nc.sync.dma_start(out=d["out"].ap()[tt * 128:(tt + 1) * 128, osl],
                                  in_=ot[:])


def build_nc(reps=1):
    import concourse.bacc as bacc
    import concourse.mybir as mybir
    import concourse.tile as tile

    f32 = mybir.dt.float32
    bf16 = mybir.dt.bfloat16
    nc = bacc.Bacc("TRN2", target_bir_lowering=False, debug=False,
                   num_devices=N_CORES)
    d = {
        "xT": nc.dram_tensor("xT", [E, TB], bf16, kind="ExternalInput"),
        "xqT": nc.dram_tensor("xqT", [EC, 128, TSH], bf16, kind="ExternalInput"),
        "wqT": nc.dram_tensor("wqT", [NP, 128, EC, 128], bf16, kind="ExternalInput"),
        "wkT": nc.dram_tensor("wkT", [E, E], bf16, kind="ExternalInput"),
        "wvT": nc.dram_tensor("wvT", [E, E], bf16, kind="ExternalInput"),
        "woT": nc.dram_tensor("woT", [E, E], bf16, kind="ExternalInput"),
        "bq_t": nc.dram_tensor("bq_t", [128, NP], f32, kind="ExternalInput"),
        "bk_t": nc.dram_tensor("bk_t", [128, NP], f32, kind="ExternalInput"),
        "bvB": nc.dram_tensor("bvB", [128, E], f32, kind="ExternalInput"),
        "boB": nc.dram_tensor("boB", [128, E], f32, kind="ExternalInput"),
        "ones": nc.dram_tensor("ones", [128, D], bf16, kind="ExternalInput"),
        "out": nc.dram_tensor("out", [TSH, E], bf16, kind="ExternalOutput"),
    }
    with tile.TileContext(nc) as tc:
        with tc.tile_pool(name="w", bufs=1) as wpool, \
             tc.tile_pool(name="big", bufs=1) as big, \
             tc.tile_pool(name="k", bufs=2) as kpool, \
             tc.tile_pool(name="s", bufs=2) as scratch:
            pools = {"w": wpool, "big": big, "k": kpool, "s": scratch}
            for _ in range(reps):
                _emit_body(nc, tc, d, pools)
    nc.compile()
    return nc


def make_in_maps(x, Wq, bq, Wk, bk, Wv, bv, Wo, bo):
    import ml_dtypes

    bf16 = ml_dtypes.bfloat16
    xT = {b: np.ascontiguousarray(x[b].T.astype(bf16)) for b in range(B)}
    wqT = np.ascontiguousarray(
        Wq.T.astype(bf16).reshape(EC, 128, NP, 128).transpose(2, 1, 0, 3))
    wkT = np.ascontiguousarray(Wk.T.astype(bf16))
    wvT = np.ascontiguousarray(Wv.T.astype(bf16))
    woT = np.ascontiguousarray(Wo.T.astype(bf16))
    bq_t = np.ascontiguousarray(bq.reshape(NP, 128).T.astype(np.float32))
    bk_t = np.ascontiguousarray(bk.reshape(NP, 128).T.astype(np.float32))
    bvB = np.ascontiguousarray(np.tile(bv.astype(np.float32), (128, 1)))
    boB = np.ascontiguousarray(np.tile(bo.astype(np.float32), (128, 1)))
    ones = np.ones((128, D), dtype=bf16)
    in_maps = []
    for c in range(N_CORES):
        b = c // (N_CORES // B)
        t0 = (c % (N_CORES // B)) * TSH
        in_maps.append({
            "xT": xT[b],
            "xqT": np.ascontiguousarray(
                xT[b][:, t0:t0 + TSH].reshape(EC, 128, TSH)),
            "wqT": wqT, "wkT": wkT, "wvT": wvT, "woT": woT,
            "bq_t": bq_t, "bk_t": bk_t, "bvB": bvB, "boB": boB,
            "ones": ones,
        })
    return in_maps


def kernel(x, Wq, bq, Wk, bk, Wv, bv, Wo, bo):
    from concourse.bass_utils import run_bass_kernel_spmd

    x = np.asarray(x, dtype=np.float32)
    args = [np.asarray(a, dtype=np.float32) for a in (Wq, bq, Wk, bk, Wv, bv, Wo, bo)]
    if "nc1" not in _NC_CACHE:
        _NC_CACHE["nc1"] = build_nc(reps=1)
    nc = _NC_CACHE["nc1"]
    in_maps = make_in_maps(x, *args)
    res = run_bass_kernel_spmd(nc, in_maps, list(range(N_CORES)))
    out = np.concatenate([res.results[c]["out"] for c in range(N_CORES)], axis=0)
    return out.reshape(B, S, E).astype(np.float32)

